# revision 21
# baseline (speedup 1.0000x reference)
"""Trainium2 Bass kernel for nn_MultiLabelClassifier.

Reference computation (B=1024, D=512, N=16, H=512):
    xb     = transpose(x, (0,2,1))                        # [B, N, D]
    h      = lrelu(xb @ Ws1 + bs1)                        # shared MLP
    shared = lrelu(h @ Ws2 + bs2)                         # [B, N, D]
    out    = sigmoid(per-branch classifier on own slice)  # [B, 1, N]
    full   = all 16 classifiers on all N*B tokens         # [N*B, 1, N]

Key identities used:
  1. `out` is the diagonal of `full`: out[b, n] = sigmoid(full[n*B+b, n]),
     so only the full pass is computed on device.
  2. The second classifier layer collapses into the first via positive
     homogeneity of leaky-relu:
         sum_h w_h * lrelu(z_h)                     (w = Wc2[n], z = x@Wc1[n]+bc1[n])
       = sum_{w_h>=0} prelu(|w_h| z_h, a=0.1)       (columns scaled by w)
       + sum_{w_h<0}  prelu(-0.1|w_h| z_h, a=10)    (columns scaled by 0.1*w)
     Host permutes the H columns of each branch's Wc1 so the w>=0 group comes
     first, and pre-scales columns; on device the H-reduction is then just the
     ScalarEngine's free-axis accumulate (accum_out) on two Prelu activations.

Sharding: pure data-parallel over batch, 128 batch rows per core, 8 cores.
Matmuls run as float32r (fp32 data, 1 cycle/row at free-dim >= 256).
"""

import os
import numpy as np

B, D, N, H = 1024, 512, 16, 512
CORES = 8
BLOC = B // CORES           # batch rows per core
T = N * BLOC                # tokens per core (token t = n*BLOC + b)
KC = D // 128               # contraction chunks
FCH = 512                   # token free-chunk for stages 1-2
NF = T // FCH               # number of free chunks
TPF = FCH // 128            # 128-token tiles per free chunk

# Stash of the last BassKernelResults (for test harness to read exec_time_ns).
LAST_RESULTS = None


def _build(npos, has_bc1, bc2, use_bf16):
    """Build the per-core Bass program (same program for every core)."""
    import concourse.bacc as bacc
    import concourse.tile as tile
    from concourse import mybir

    f32 = mybir.dt.float32
    f32r = mybir.dt.bfloat16 if use_bf16 else mybir.dt.float32r
    bf = mybir.dt.bfloat16 if use_bf16 else mybir.dt.float32
    A = mybir.ActivationFunctionType

    has_bc2 = bool(np.any(bc2 != 0.0))

    pos_max = max(1, int(max(npos)))
    neg_max = max(1, int(H - min(npos)))

    nc = bacc.Bacc(trn_type="TRN2")

    xs_d = nc.dram_tensor("xs", [128, KC, T], f32r, kind="ExternalInput")
    ws1_d = nc.dram_tensor("ws1", [128, KC, KC, 128], f32r, kind="ExternalInput")
    ws2_d = nc.dram_tensor("ws2", [128, KC, KC, 128], f32r, kind="ExternalInput")
    bs1_d = nc.dram_tensor("bs1", [128, KC], f32, kind="ExternalInput")
    bs2_d = nc.dram_tensor("bs2", [128, KC], f32, kind="ExternalInput")
    wc_d = nc.dram_tensor("wc", [128, N, KC, H], f32r, kind="ExternalInput")
    if has_bc1:
        bc1_d = nc.dram_tensor("bc1m", [N, H], f32, kind="ExternalInput")
    flog_d = nc.dram_tensor("flog", [T, N], f32, kind="ExternalOutput")
    probs_d = nc.dram_tensor("probs", [BLOC, N], f32, kind="ExternalOutput")

    with tile.TileContext(nc) as tc:
        with (
            tc.tile_pool(name="consts", bufs=1) as consts,
            tc.tile_pool(name="xs", bufs=4) as xs_pool,
            tc.tile_pool(name="h", bufs=1) as h_pool,
            tc.tile_pool(name="sh", bufs=2) as sh_pool,
            tc.tile_pool(name="scr", bufs=1) as scr_pool,
            tc.tile_pool(name="dv", bufs=1) as dv_pool,
            tc.tile_pool(name="acc", bufs=12) as acc_pool,
            tc.tile_pool(name="zed", bufs=2) as z_pool,
            tc.tile_pool(name="outs", bufs=1) as out_pool,
            tc.tile_pool(name="ps12", bufs=2, space="PSUM") as ps12,
            tc.tile_pool(name="ps3", bufs=6, space="PSUM") as ps3,
        ):
            xs_tiles = []
            for f in range(NF):
                xs_t = xs_pool.tile([128, KC, FCH], f32r)
                nc.sync.dma_start(
                    out=xs_t[:], in_=xs_d[:, :, f * FCH:(f + 1) * FCH])
                xs_tiles.append(xs_t)
            ws1_sb = consts.tile([128, KC, KC, 128], f32r)
            nc.sync.dma_start(out=ws1_sb[:], in_=ws1_d[:])
            ws2_sb = consts.tile([128, KC, KC, 128], f32r)
            nc.sync.dma_start(out=ws2_sb[:], in_=ws2_d[:])
            bs1_sb = consts.tile([128, KC], f32)
            nc.sync.dma_start(out=bs1_sb[:], in_=bs1_d[:])
            bs2_sb = consts.tile([128, KC], f32)
            nc.sync.dma_start(out=bs2_sb[:], in_=bs2_d[:])
            if has_bc1:
                bc1_sb = consts.tile([128, N, H], f32)
                nc.sync.dma_start(
                    out=bc1_sb[:], in_=bc1_d[:].partition_broadcast(128)
                )
            wc_sb = consts.tile([128, N, KC, H], f32r)
            for n in range(N):
                nc.sync.dma_start(out=wc_sb[:, n], in_=wc_d[:, n])

            logits_sb = out_pool.tile([128, N, N], f32)   # [b, ttile, n]
            probs_sb = out_pool.tile([128, N], f32)

            for f in range(NF):
                xs_t = xs_tiles[f]

                # stage 1: h = prelu(Ws1^T x + bs1)
                h_t = h_pool.tile([128, KC, FCH], f32r)
                for oi in range(KC):
                    ps = ps12.tile([128, FCH], f32)
                    for ki in range(KC):
                        nc.tensor.matmul(
                            ps[:],
                            ws1_sb[:, ki, oi, :],
                            xs_t[:, ki, :],
                            start=(ki == 0),
                            stop=(ki == KC - 1),
                        )
                    nc.scalar.activation(
                        h_t[:, oi, :], ps[:], A.Prelu,
                        bias=bs1_sb[:, oi:oi + 1], scale=1.0, alpha=0.1,
                    )

                # stage 2: shared = prelu(Ws2^T h + bs2)
                sh_t = sh_pool.tile([128, KC, FCH], f32r)
                for oi in range(KC):
                    ps = ps12.tile([128, FCH], f32)
                    for ki in range(KC):
                        nc.tensor.matmul(
                            ps[:],
                            ws2_sb[:, ki, oi, :],
                            h_t[:, ki, :],
                            start=(ki == 0),
                            stop=(ki == KC - 1),
                        )
                    nc.scalar.activation(
                        sh_t[:, oi, :], ps[:], A.Prelu,
                        bias=bs2_sb[:, oi:oi + 1], scale=1.0, alpha=0.1,
                    )

                # stage 3: all 16 classifiers on this chunk's 4 token-tiles
                for tt in range(TPF):
                    t = f * TPF + tt          # global token tile == branch id
                    c0 = tt * 128
                    for n in range(N):
                        ps = ps3.tile([128, H], f32)
                        for ki in range(KC):
                            nc.tensor.matmul(
                                ps[:],
                                sh_t[:, ki, c0:c0 + 128],
                                wc_sb[:, n, ki, :],
                                start=(ki == 0),
                                stop=(ki == KC - 1),
                            )
                        if has_bc1:
                            z_t = z_pool.tile([128, H], f32)
                            nc.vector.tensor_add(z_t[:], ps[:], bc1_sb[:, n, :])
                            src = z_t
                        else:
                            src = ps
                        npn = int(npos[n])
                        # Evacuate the whole psum tile to bf16 SBUF on
                        # ScalarE (one Copy, no accumulator), then reduce
                        # both halves on VectorE at 2x bf16 rate:
                        #   pos:  |w|lrelu(z) == max(p, 0.1p)   p =  |w| z
                        #   neg: -|w|lrelu(z) == min(u, 10u)    u = -.1|w| z
                        ucp = dv_pool.tile([128, H], bf, tag="ucp")
                        nc.scalar.activation(ucp[:], src[:], A.Copy)
                        a_p = acc_pool.tile([128, 1], f32, tag="acc")
                        dsc = dv_pool.tile([128, H], bf, tag="dscr")
                        if npn > 0:
                            nc.vector.scalar_tensor_tensor(
                                out=dsc[:, :npn],
                                in0=ucp[:, :npn],
                                scalar=0.1,
                                in1=ucp[:, :npn],
                                op0=mybir.AluOpType.mult,
                                op1=mybir.AluOpType.max,
                                accum_out=a_p[:],
                            )
                        else:
                            nc.vector.memset(a_p[:], 0.0)
                        if npn < H:
                            nm = H - npn
                            a_m = acc_pool.tile([128, 1], f32, tag="acc")
                            nc.vector.scalar_tensor_tensor(
                                out=dsc[:, npn:],
                                in0=ucp[:, npn:],
                                scalar=10.0,
                                in1=ucp[:, npn:],
                                op0=mybir.AluOpType.mult,
                                op1=mybir.AluOpType.min,
                                accum_out=a_m[:],
                            )
                            nc.vector.tensor_add(
                                logits_sb[:, t, n:n + 1], a_p[:], a_m[:])
                        else:
                            nc.vector.tensor_copy(
                                logits_sb[:, t, n:n + 1], a_p[:])
                        if has_bc2:
                            nc.vector.tensor_scalar(
                                out=logits_sb[:, t, n:n + 1],
                                in0=logits_sb[:, t, n:n + 1],
                                scalar1=float(bc2[n]),
                                scalar2=None,
                                op0=mybir.AluOpType.add,
                            )
                    nc.scalar.activation(
                        probs_sb[:, t:t + 1], logits_sb[:, t, t:t + 1],
                        A.Sigmoid,
                    )
                    nc.sync.dma_start(
                        out=flog_d[t * 128:(t + 1) * 128, :],
                        in_=logits_sb[:, t, :],
                    )
            nc.sync.dma_start(out=probs_d[:], in_=probs_sb[:])

    nc.compile()
    return nc


def kernel(x, Ws1, bs1, Ws2, bs2, Wc1, bc1, Wc2, bc2):
    global LAST_RESULTS
    from concourse.bass_utils import run_bass_kernel_spmd

    x = np.ascontiguousarray(np.asarray(x, dtype=np.float32))
    Ws1 = np.asarray(Ws1, dtype=np.float32)
    bs1 = np.asarray(bs1, dtype=np.float32)
    Ws2 = np.asarray(Ws2, dtype=np.float32)
    bs2 = np.asarray(bs2, dtype=np.float32)
    Wc1 = np.asarray(Wc1, dtype=np.float32)
    bc1 = np.asarray(bc1, dtype=np.float32)
    Wc2 = np.asarray(Wc2, dtype=np.float32)
    bc2 = np.asarray(bc2, dtype=np.float32)

    # ---- host-side weight preprocessing (O(weights) only) ----
    # Per branch: put w>=0 columns first, scale pos cols by w, neg by 0.1*w
    # (see module docstring identity 2).
    npos = np.zeros(N, dtype=np.int64)
    wc_mod = np.empty_like(Wc1)               # [N, D, H]
    bc1_mod = np.empty_like(bc1)              # [N, H]
    for n in range(N):
        w = Wc2[n]
        pos = np.flatnonzero(w >= 0.0)
        neg = np.flatnonzero(w < 0.0)
        npos[n] = len(pos)
        scale = np.concatenate([w[pos], 0.1 * w[neg]])   # neg w<0 -> -0.1|w|
        perm = np.concatenate([pos, neg])
        wc_mod[n] = Wc1[n][:, perm] * scale[None, :]
        bc1_mod[n] = bc1[n][perm] * scale

    has_bc1 = bool(np.any(bc1_mod != 0.0))

    # ---- reshape to device layouts ----
    ws1_h = np.ascontiguousarray(
        Ws1.reshape(KC, 128, KC, 128).transpose(1, 0, 2, 3))
    ws2_h = np.ascontiguousarray(
        Ws2.reshape(KC, 128, KC, 128).transpose(1, 0, 2, 3))
    bs1_h = np.ascontiguousarray(bs1.reshape(KC, 128).T)
    bs2_h = np.ascontiguousarray(bs2.reshape(KC, 128).T)
    wc_h = np.ascontiguousarray(
        wc_mod.reshape(N, KC, 128, H).transpose(2, 0, 1, 3))

    use_bf16 = os.environ.get("KERNEL_DTYPE", "bf16") == "bf16"
    nc = _build(npos, has_bc1, bc2, use_bf16)
    if use_bf16:
        import ml_dtypes
        mmdt = ml_dtypes.bfloat16
    else:
        mmdt = np.float32

    in_maps = []
    for c in range(CORES):
        xc = x[c * BLOC:(c + 1) * BLOC]                   # [128, D, N]
        # token t = n*BLOC + b ; xs[kp, ki, t] = x[b, ki*128+kp, n]
        xs_h = np.ascontiguousarray(
            xc.transpose(1, 2, 0)                          # [D, N, BLOC]
            .reshape(KC, 128, T)
            .transpose(1, 0, 2))
        m = {
            "xs": xs_h.astype(mmdt),
            "ws1": ws1_h.astype(mmdt), "ws2": ws2_h.astype(mmdt),
            "bs1": bs1_h, "bs2": bs2_h,
            "wc": wc_h.astype(mmdt),
        }
        if has_bc1:
            m["bc1m"] = np.ascontiguousarray(bc1_mod)
        in_maps.append(m)

    res = run_bass_kernel_spmd(
        nc, in_maps, core_ids=list(range(CORES)),
        trace=bool(int(os.environ.get("KERNEL_TRACE", "0"))),
    )
    LAST_RESULTS = res

    out = np.empty((B, 1, N), dtype=np.float32)
    full = np.empty((N * B, 1, N), dtype=np.float32)
    fullv = full.reshape(N, CORES, BLOC, N)
    for c in range(CORES):
        r = res.results[c]
        out[c * BLOC:(c + 1) * BLOC, 0, :] = r["probs"]
        fullv[:, c, :, :] = r["flog"].reshape(N, BLOC, N)
    return out, full


# revision 22
# speedup vs baseline: 1.5749x; 1.5749x over previous
"""Trainium2 Bass kernel for nn_MultiLabelClassifier.

Reference computation (B=1024, D=512, N=16, H=512):
    xb     = transpose(x, (0,2,1))                        # [B, N, D]
    h      = lrelu(xb @ Ws1 + bs1)                        # shared MLP
    shared = lrelu(h @ Ws2 + bs2)                         # [B, N, D]
    out    = sigmoid(per-branch classifier on own slice)  # [B, 1, N]
    full   = all 16 classifiers on all N*B tokens         # [N*B, 1, N]

Key identities used:
  1. `out` is the diagonal of `full`: out[b, n] = sigmoid(full[n*B+b, n]),
     so only the full pass is computed on device.
  2. The second classifier layer collapses into the first via positive
     homogeneity of leaky-relu:
         sum_h w_h * lrelu(z_h)                     (w = Wc2[n], z = x@Wc1[n]+bc1[n])
       = sum_{w_h>=0} prelu(|w_h| z_h, a=0.1)       (columns scaled by w)
       + sum_{w_h<0}  prelu(-0.1|w_h| z_h, a=10)    (columns scaled by 0.1*w)
     Host permutes the H columns of each branch's Wc1 so the w>=0 group comes
     first, and pre-scales columns; on device the H-reduction is then just the
     ScalarEngine's free-axis accumulate (accum_out) on two Prelu activations.

Sharding: pure data-parallel over batch, 128 batch rows per core, 8 cores.
Matmuls run as float32r (fp32 data, 1 cycle/row at free-dim >= 256).
"""

import os
import numpy as np

B, D, N, H = 1024, 512, 16, 512
CORES = 8
BLOC = B // CORES           # batch rows per core
T = N * BLOC                # tokens per core (token t = n*BLOC + b)
KC = D // 128               # contraction chunks
FCH = 512                   # token free-chunk for stages 1-2
NF = T // FCH               # number of free chunks
TPF = FCH // 128            # 128-token tiles per free chunk

# Stash of the last BassKernelResults (for test harness to read exec_time_ns).
LAST_RESULTS = None


def _build(npos, has_bc1, bc2, use_bf16):
    """Build the per-core Bass program (same program for every core)."""
    import concourse.bacc as bacc
    import concourse.tile as tile
    from concourse import mybir

    f32 = mybir.dt.float32
    f32r = mybir.dt.bfloat16 if use_bf16 else mybir.dt.float32r
    bf = mybir.dt.bfloat16 if use_bf16 else mybir.dt.float32
    A = mybir.ActivationFunctionType

    has_bc2 = bool(np.any(bc2 != 0.0))

    pos_max = max(1, int(max(npos)))
    neg_max = max(1, int(H - min(npos)))

    nc = bacc.Bacc(trn_type="TRN2")

    xs_d = nc.dram_tensor("xs", [128, KC, T], f32r, kind="ExternalInput")
    ws1_d = nc.dram_tensor("ws1", [128, KC, KC, 128], f32r, kind="ExternalInput")
    ws2_d = nc.dram_tensor("ws2", [128, KC, KC, 128], f32r, kind="ExternalInput")
    bs1_d = nc.dram_tensor("bs1", [128, KC], f32, kind="ExternalInput")
    bs2_d = nc.dram_tensor("bs2", [128, KC], f32, kind="ExternalInput")
    wc_d = nc.dram_tensor("wc", [128, N, KC, H], f32r, kind="ExternalInput")
    if has_bc1:
        bc1_d = nc.dram_tensor("bc1m", [N, H], f32, kind="ExternalInput")
    flog_d = nc.dram_tensor("flog", [T, N], f32, kind="ExternalOutput")
    probs_d = nc.dram_tensor("probs", [BLOC, N], f32, kind="ExternalOutput")

    with tile.TileContext(nc) as tc:
        with (
            tc.tile_pool(name="consts", bufs=1) as consts,
            tc.tile_pool(name="xs", bufs=4) as xs_pool,
            tc.tile_pool(name="h", bufs=1) as h_pool,
            tc.tile_pool(name="sh", bufs=2) as sh_pool,
            tc.tile_pool(name="scr", bufs=1) as scr_pool,
            tc.tile_pool(name="dv", bufs=4) as dv_pool,
            tc.tile_pool(name="acc", bufs=12) as acc_pool,
            tc.tile_pool(name="zed", bufs=2) as z_pool,
            tc.tile_pool(name="outs", bufs=1) as out_pool,
            tc.tile_pool(name="ps12", bufs=2, space="PSUM") as ps12,
            tc.tile_pool(name="ps3", bufs=6, space="PSUM") as ps3,
        ):
            xs_tiles = []
            for f in range(NF):
                xs_t = xs_pool.tile([128, KC, FCH], f32r)
                nc.sync.dma_start(
                    out=xs_t[:], in_=xs_d[:, :, f * FCH:(f + 1) * FCH])
                xs_tiles.append(xs_t)
            ws1_sb = consts.tile([128, KC, KC, 128], f32r)
            nc.sync.dma_start(out=ws1_sb[:], in_=ws1_d[:])
            ws2_sb = consts.tile([128, KC, KC, 128], f32r)
            nc.sync.dma_start(out=ws2_sb[:], in_=ws2_d[:])
            bs1_sb = consts.tile([128, KC], f32)
            nc.sync.dma_start(out=bs1_sb[:], in_=bs1_d[:])
            bs2_sb = consts.tile([128, KC], f32)
            nc.sync.dma_start(out=bs2_sb[:], in_=bs2_d[:])
            if has_bc1:
                bc1_sb = consts.tile([128, N, H], f32)
                nc.sync.dma_start(
                    out=bc1_sb[:], in_=bc1_d[:].partition_broadcast(128)
                )
            wc_sb = consts.tile([128, N, KC, H], f32r)
            for n in range(N):
                nc.sync.dma_start(out=wc_sb[:, n], in_=wc_d[:, n])

            logits_sb = out_pool.tile([128, N, N], f32)   # [b, ttile, n]
            probs_sb = out_pool.tile([128, N], f32)

            for f in range(NF):
                xs_t = xs_tiles[f]

                # stage 1: h = prelu(Ws1^T x + bs1)
                h_t = h_pool.tile([128, KC, FCH], f32r)
                for oi in range(KC):
                    ps = ps12.tile([128, FCH], f32)
                    for ki in range(KC):
                        nc.tensor.matmul(
                            ps[:],
                            ws1_sb[:, ki, oi, :],
                            xs_t[:, ki, :],
                            start=(ki == 0),
                            stop=(ki == KC - 1),
                        )
                    nc.scalar.activation(
                        h_t[:, oi, :], ps[:], A.Prelu,
                        bias=bs1_sb[:, oi:oi + 1], scale=1.0, alpha=0.1,
                    )

                # stage 2: shared = prelu(Ws2^T h + bs2)
                sh_t = sh_pool.tile([128, KC, FCH], f32r)
                for oi in range(KC):
                    ps = ps12.tile([128, FCH], f32)
                    for ki in range(KC):
                        nc.tensor.matmul(
                            ps[:],
                            ws2_sb[:, ki, oi, :],
                            h_t[:, ki, :],
                            start=(ki == 0),
                            stop=(ki == KC - 1),
                        )
                    nc.scalar.activation(
                        sh_t[:, oi, :], ps[:], A.Prelu,
                        bias=bs2_sb[:, oi:oi + 1], scale=1.0, alpha=0.1,
                    )

                # stage 3: all 16 classifiers on this chunk's 4 token-tiles
                for tt in range(TPF):
                    t = f * TPF + tt          # global token tile == branch id
                    c0 = tt * 128
                    for n in range(N):
                        ps = ps3.tile([128, H], f32)
                        for ki in range(KC):
                            nc.tensor.matmul(
                                ps[:],
                                sh_t[:, ki, c0:c0 + 128],
                                wc_sb[:, n, ki, :],
                                start=(ki == 0),
                                stop=(ki == KC - 1),
                            )
                        if has_bc1:
                            z_t = z_pool.tile([128, H], f32)
                            nc.vector.tensor_add(z_t[:], ps[:], bc1_sb[:, n, :])
                            src = z_t
                        else:
                            src = ps
                        npn = int(npos[n])
                        # Evacuate the whole psum tile to bf16 SBUF on
                        # ScalarE (one Copy, no accumulator), then reduce
                        # both halves on VectorE at 2x bf16 rate:
                        #   pos:  |w|lrelu(z) == max(p, 0.1p)   p =  |w| z
                        #   neg: -|w|lrelu(z) == min(u, 10u)    u = -.1|w| z
                        ucp = dv_pool.tile([128, H], bf, tag="ucp")
                        nc.scalar.activation(ucp[:], src[:], A.Copy)
                        a_p = acc_pool.tile([128, 1], f32, tag="acc")
                        dsc = dv_pool.tile([128, H], bf, tag="dscr")
                        if npn > 0:
                            nc.vector.scalar_tensor_tensor(
                                out=dsc[:, :npn],
                                in0=ucp[:, :npn],
                                scalar=0.1,
                                in1=ucp[:, :npn],
                                op0=mybir.AluOpType.mult,
                                op1=mybir.AluOpType.max,
                                accum_out=a_p[:],
                            )
                        else:
                            nc.vector.memset(a_p[:], 0.0)
                        if npn < H:
                            nm = H - npn
                            a_m = acc_pool.tile([128, 1], f32, tag="acc")
                            nc.vector.scalar_tensor_tensor(
                                out=dsc[:, npn:],
                                in0=ucp[:, npn:],
                                scalar=10.0,
                                in1=ucp[:, npn:],
                                op0=mybir.AluOpType.mult,
                                op1=mybir.AluOpType.min,
                                accum_out=a_m[:],
                            )
                            nc.vector.tensor_add(
                                logits_sb[:, t, n:n + 1], a_p[:], a_m[:])
                        else:
                            nc.vector.tensor_copy(
                                logits_sb[:, t, n:n + 1], a_p[:])
                        if has_bc2:
                            nc.vector.tensor_scalar(
                                out=logits_sb[:, t, n:n + 1],
                                in0=logits_sb[:, t, n:n + 1],
                                scalar1=float(bc2[n]),
                                scalar2=None,
                                op0=mybir.AluOpType.add,
                            )
                    nc.scalar.activation(
                        probs_sb[:, t:t + 1], logits_sb[:, t, t:t + 1],
                        A.Sigmoid,
                    )
                    nc.sync.dma_start(
                        out=flog_d[t * 128:(t + 1) * 128, :],
                        in_=logits_sb[:, t, :],
                    )
            nc.sync.dma_start(out=probs_d[:], in_=probs_sb[:])

    nc.compile()
    return nc


def kernel(x, Ws1, bs1, Ws2, bs2, Wc1, bc1, Wc2, bc2):
    global LAST_RESULTS
    from concourse.bass_utils import run_bass_kernel_spmd

    x = np.ascontiguousarray(np.asarray(x, dtype=np.float32))
    Ws1 = np.asarray(Ws1, dtype=np.float32)
    bs1 = np.asarray(bs1, dtype=np.float32)
    Ws2 = np.asarray(Ws2, dtype=np.float32)
    bs2 = np.asarray(bs2, dtype=np.float32)
    Wc1 = np.asarray(Wc1, dtype=np.float32)
    bc1 = np.asarray(bc1, dtype=np.float32)
    Wc2 = np.asarray(Wc2, dtype=np.float32)
    bc2 = np.asarray(bc2, dtype=np.float32)

    # ---- host-side weight preprocessing (O(weights) only) ----
    # Per branch: put w>=0 columns first, scale pos cols by w, neg by 0.1*w
    # (see module docstring identity 2).
    npos = np.zeros(N, dtype=np.int64)
    wc_mod = np.empty_like(Wc1)               # [N, D, H]
    bc1_mod = np.empty_like(bc1)              # [N, H]
    for n in range(N):
        w = Wc2[n]
        pos = np.flatnonzero(w >= 0.0)
        neg = np.flatnonzero(w < 0.0)
        npos[n] = len(pos)
        scale = np.concatenate([w[pos], 0.1 * w[neg]])   # neg w<0 -> -0.1|w|
        perm = np.concatenate([pos, neg])
        wc_mod[n] = Wc1[n][:, perm] * scale[None, :]
        bc1_mod[n] = bc1[n][perm] * scale

    has_bc1 = bool(np.any(bc1_mod != 0.0))

    # ---- reshape to device layouts ----
    ws1_h = np.ascontiguousarray(
        Ws1.reshape(KC, 128, KC, 128).transpose(1, 0, 2, 3))
    ws2_h = np.ascontiguousarray(
        Ws2.reshape(KC, 128, KC, 128).transpose(1, 0, 2, 3))
    bs1_h = np.ascontiguousarray(bs1.reshape(KC, 128).T)
    bs2_h = np.ascontiguousarray(bs2.reshape(KC, 128).T)
    wc_h = np.ascontiguousarray(
        wc_mod.reshape(N, KC, 128, H).transpose(2, 0, 1, 3))

    use_bf16 = os.environ.get("KERNEL_DTYPE", "bf16") == "bf16"
    nc = _build(npos, has_bc1, bc2, use_bf16)
    if use_bf16:
        import ml_dtypes
        mmdt = ml_dtypes.bfloat16
    else:
        mmdt = np.float32

    in_maps = []
    for c in range(CORES):
        xc = x[c * BLOC:(c + 1) * BLOC]                   # [128, D, N]
        # token t = n*BLOC + b ; xs[kp, ki, t] = x[b, ki*128+kp, n]
        xs_h = np.ascontiguousarray(
            xc.transpose(1, 2, 0)                          # [D, N, BLOC]
            .reshape(KC, 128, T)
            .transpose(1, 0, 2))
        m = {
            "xs": xs_h.astype(mmdt),
            "ws1": ws1_h.astype(mmdt), "ws2": ws2_h.astype(mmdt),
            "bs1": bs1_h, "bs2": bs2_h,
            "wc": wc_h.astype(mmdt),
        }
        if has_bc1:
            m["bc1m"] = np.ascontiguousarray(bc1_mod)
        in_maps.append(m)

    res = run_bass_kernel_spmd(
        nc, in_maps, core_ids=list(range(CORES)),
        trace=bool(int(os.environ.get("KERNEL_TRACE", "0"))),
    )
    LAST_RESULTS = res

    out = np.empty((B, 1, N), dtype=np.float32)
    full = np.empty((N * B, 1, N), dtype=np.float32)
    fullv = full.reshape(N, CORES, BLOC, N)
    for c in range(CORES):
        r = res.results[c]
        out[c * BLOC:(c + 1) * BLOC, 0, :] = r["probs"]
        fullv[:, c, :, :] = r["flog"].reshape(N, BLOC, N)
    return out, full


# revision 23
# speedup vs baseline: 1.5812x; 1.0040x over previous
"""Trainium2 Bass kernel for nn_MultiLabelClassifier.

Reference computation (B=1024, D=512, N=16, H=512):
    xb     = transpose(x, (0,2,1))                        # [B, N, D]
    h      = lrelu(xb @ Ws1 + bs1)                        # shared MLP
    shared = lrelu(h @ Ws2 + bs2)                         # [B, N, D]
    out    = sigmoid(per-branch classifier on own slice)  # [B, 1, N]
    full   = all 16 classifiers on all N*B tokens         # [N*B, 1, N]

Key identities used:
  1. `out` is the diagonal of `full`: out[b, n] = sigmoid(full[n*B+b, n]),
     so only the full pass is computed on device.
  2. The second classifier layer collapses into the first via positive
     homogeneity of leaky-relu:
         sum_h w_h * lrelu(z_h)                     (w = Wc2[n], z = x@Wc1[n]+bc1[n])
       = sum_{w_h>=0} prelu(|w_h| z_h, a=0.1)       (columns scaled by w)
       + sum_{w_h<0}  prelu(-0.1|w_h| z_h, a=10)    (columns scaled by 0.1*w)
     Host permutes the H columns of each branch's Wc1 so the w>=0 group comes
     first, and pre-scales columns; on device the H-reduction is then just the
     ScalarEngine's free-axis accumulate (accum_out) on two Prelu activations.

Sharding: pure data-parallel over batch, 128 batch rows per core, 8 cores.
Matmuls run as float32r (fp32 data, 1 cycle/row at free-dim >= 256).
"""

import os
import numpy as np

B, D, N, H = 1024, 512, 16, 512
CORES = 8
BLOC = B // CORES           # batch rows per core
T = N * BLOC                # tokens per core (token t = n*BLOC + b)
KC = D // 128               # contraction chunks
FCH = 512                   # token free-chunk for stages 1-2
NF = T // FCH               # number of free chunks
TPF = FCH // 128            # 128-token tiles per free chunk

# Stash of the last BassKernelResults (for test harness to read exec_time_ns).
LAST_RESULTS = None


def _build(npos, has_bc1, bc2, use_bf16):
    """Build the per-core Bass program (same program for every core)."""
    import concourse.bacc as bacc
    import concourse.tile as tile
    from concourse import mybir

    f32 = mybir.dt.float32
    f32r = mybir.dt.bfloat16 if use_bf16 else mybir.dt.float32r
    bf = mybir.dt.bfloat16 if use_bf16 else mybir.dt.float32
    A = mybir.ActivationFunctionType

    has_bc2 = bool(np.any(bc2 != 0.0))

    pos_max = max(1, int(max(npos)))
    neg_max = max(1, int(H - min(npos)))

    nc = bacc.Bacc(trn_type="TRN2")

    xs_d = nc.dram_tensor("xs", [128, KC, T], f32r, kind="ExternalInput")
    ws1_d = nc.dram_tensor("ws1", [128, KC, KC, 128], f32r, kind="ExternalInput")
    ws2_d = nc.dram_tensor("ws2", [128, KC, KC, 128], f32r, kind="ExternalInput")
    bs1_d = nc.dram_tensor("bs1", [128, KC], f32, kind="ExternalInput")
    bs2_d = nc.dram_tensor("bs2", [128, KC], f32, kind="ExternalInput")
    wc_d = nc.dram_tensor("wc", [128, N, KC, H], f32r, kind="ExternalInput")
    if has_bc1:
        bc1_d = nc.dram_tensor("bc1m", [N, H], f32, kind="ExternalInput")
    flog_d = nc.dram_tensor("flog", [T, N], f32, kind="ExternalOutput")
    probs_d = nc.dram_tensor("probs", [BLOC, N], f32, kind="ExternalOutput")

    with tile.TileContext(nc) as tc:
        with (
            tc.tile_pool(name="consts", bufs=1) as consts,
            tc.tile_pool(name="xs", bufs=4) as xs_pool,
            tc.tile_pool(name="h", bufs=1) as h_pool,
            tc.tile_pool(name="sh", bufs=2) as sh_pool,
            tc.tile_pool(name="scr", bufs=1) as scr_pool,
            tc.tile_pool(name="dv", bufs=4) as dv_pool,
            tc.tile_pool(name="acc", bufs=12) as acc_pool,
            tc.tile_pool(name="zed", bufs=2) as z_pool,
            tc.tile_pool(name="outs", bufs=1) as out_pool,
            tc.tile_pool(name="ps12", bufs=2, space="PSUM") as ps12,
            tc.tile_pool(name="ps3", bufs=6, space="PSUM") as ps3,
        ):
            xs_tiles = []
            for f in range(NF):
                xs_t = xs_pool.tile([128, KC, FCH], f32r)
                nc.sync.dma_start(
                    out=xs_t[:], in_=xs_d[:, :, f * FCH:(f + 1) * FCH])
                xs_tiles.append(xs_t)
            ws1_sb = consts.tile([128, KC, KC, 128], f32r)
            nc.sync.dma_start(out=ws1_sb[:], in_=ws1_d[:])
            ws2_sb = consts.tile([128, KC, KC, 128], f32r)
            nc.sync.dma_start(out=ws2_sb[:], in_=ws2_d[:])
            bs1_sb = consts.tile([128, KC], f32)
            nc.sync.dma_start(out=bs1_sb[:], in_=bs1_d[:])
            bs2_sb = consts.tile([128, KC], f32)
            nc.sync.dma_start(out=bs2_sb[:], in_=bs2_d[:])
            if has_bc1:
                bc1_sb = consts.tile([128, N, H], f32)
                nc.sync.dma_start(
                    out=bc1_sb[:], in_=bc1_d[:].partition_broadcast(128)
                )
            wc_sb = consts.tile([128, N, KC, H], f32r)
            for n in range(N):
                nc.sync.dma_start(out=wc_sb[:, n], in_=wc_d[:, n])

            logits_sb = out_pool.tile([128, N, N], f32)   # [b, ttile, n]
            probs_sb = out_pool.tile([128, N], f32)

            for f in range(NF):
                xs_t = xs_tiles[f]

                # stage 1: h = prelu(Ws1^T x + bs1)
                h_t = h_pool.tile([128, KC, FCH], f32r)
                for oi in range(KC):
                    ps = ps12.tile([128, FCH], f32)
                    for ki in range(KC):
                        nc.tensor.matmul(
                            ps[:],
                            ws1_sb[:, ki, oi, :],
                            xs_t[:, ki, :],
                            start=(ki == 0),
                            stop=(ki == KC - 1),
                        )
                    nc.scalar.activation(
                        h_t[:, oi, :], ps[:], A.Prelu,
                        bias=bs1_sb[:, oi:oi + 1], scale=1.0, alpha=0.1,
                    )

                # stage 2: shared = prelu(Ws2^T h + bs2)
                sh_t = sh_pool.tile([128, KC, FCH], f32r)
                for oi in range(KC):
                    ps = ps12.tile([128, FCH], f32)
                    for ki in range(KC):
                        nc.tensor.matmul(
                            ps[:],
                            ws2_sb[:, ki, oi, :],
                            h_t[:, ki, :],
                            start=(ki == 0),
                            stop=(ki == KC - 1),
                        )
                    nc.scalar.activation(
                        sh_t[:, oi, :], ps[:], A.Prelu,
                        bias=bs2_sb[:, oi:oi + 1], scale=1.0, alpha=0.1,
                    )

                # stage 3: all 16 classifiers on this chunk's 4 token-tiles
                for tt in range(TPF):
                    t = f * TPF + tt          # global token tile == branch id
                    c0 = tt * 128
                    for n in range(N):
                        ps = ps3.tile([128, H], f32)
                        for ki in range(KC):
                            nc.tensor.matmul(
                                ps[:],
                                sh_t[:, ki, c0:c0 + 128],
                                wc_sb[:, n, ki, :],
                                start=(ki == 0),
                                stop=(ki == KC - 1),
                            )
                        if has_bc1:
                            z_t = z_pool.tile([128, H], f32)
                            nc.vector.tensor_add(z_t[:], ps[:], bc1_sb[:, n, :])
                            src = z_t
                        else:
                            src = ps
                        npn = int(npos[n])
                        # ScalarE transforms each half in place of the evac:
                        #   pos:  |w|lrelu(z) == prelu(p, 0.1)   p =  |w| z
                        #   neg: -|w|lrelu(z) == prelu(u, 10)    u = -.1|w| z
                        # Signs are folded into the columns, so the logit is
                        # a single plain sum over all 512 transformed values
                        # (one VectorE tensor_reduce, fp32 out).
                        ucp = dv_pool.tile([128, H], bf, tag="ucp")
                        if npn > 0:
                            nc.scalar.activation(
                                ucp[:, :npn], src[:, :npn], A.Prelu,
                                bias=0.0, scale=1.0, alpha=0.1)
                        if npn < H:
                            nc.scalar.activation(
                                ucp[:, npn:], src[:, npn:], A.Prelu,
                                bias=0.0, scale=1.0, alpha=10.0)
                        nc.vector.tensor_reduce(
                            out=logits_sb[:, t, n:n + 1],
                            in_=ucp[:],
                            axis=mybir.AxisListType.X,
                            op=mybir.AluOpType.add,
                        )
                        if has_bc2:
                            nc.vector.tensor_scalar(
                                out=logits_sb[:, t, n:n + 1],
                                in0=logits_sb[:, t, n:n + 1],
                                scalar1=float(bc2[n]),
                                scalar2=None,
                                op0=mybir.AluOpType.add,
                            )
                    nc.scalar.activation(
                        probs_sb[:, t:t + 1], logits_sb[:, t, t:t + 1],
                        A.Sigmoid,
                    )
                    nc.sync.dma_start(
                        out=flog_d[t * 128:(t + 1) * 128, :],
                        in_=logits_sb[:, t, :],
                    )
            nc.sync.dma_start(out=probs_d[:], in_=probs_sb[:])

    nc.compile()
    return nc


def kernel(x, Ws1, bs1, Ws2, bs2, Wc1, bc1, Wc2, bc2):
    global LAST_RESULTS
    from concourse.bass_utils import run_bass_kernel_spmd

    x = np.ascontiguousarray(np.asarray(x, dtype=np.float32))
    Ws1 = np.asarray(Ws1, dtype=np.float32)
    bs1 = np.asarray(bs1, dtype=np.float32)
    Ws2 = np.asarray(Ws2, dtype=np.float32)
    bs2 = np.asarray(bs2, dtype=np.float32)
    Wc1 = np.asarray(Wc1, dtype=np.float32)
    bc1 = np.asarray(bc1, dtype=np.float32)
    Wc2 = np.asarray(Wc2, dtype=np.float32)
    bc2 = np.asarray(bc2, dtype=np.float32)

    # ---- host-side weight preprocessing (O(weights) only) ----
    # Per branch: put w>=0 columns first, scale pos cols by w, neg by 0.1*w
    # (see module docstring identity 2).
    npos = np.zeros(N, dtype=np.int64)
    wc_mod = np.empty_like(Wc1)               # [N, D, H]
    bc1_mod = np.empty_like(bc1)              # [N, H]
    for n in range(N):
        w = Wc2[n]
        pos = np.flatnonzero(w >= 0.0)
        neg = np.flatnonzero(w < 0.0)
        npos[n] = len(pos)
        scale = np.concatenate([w[pos], 0.1 * w[neg]])   # neg w<0 -> -0.1|w|
        perm = np.concatenate([pos, neg])
        wc_mod[n] = Wc1[n][:, perm] * scale[None, :]
        bc1_mod[n] = bc1[n][perm] * scale

    has_bc1 = bool(np.any(bc1_mod != 0.0))

    # ---- reshape to device layouts ----
    ws1_h = np.ascontiguousarray(
        Ws1.reshape(KC, 128, KC, 128).transpose(1, 0, 2, 3))
    ws2_h = np.ascontiguousarray(
        Ws2.reshape(KC, 128, KC, 128).transpose(1, 0, 2, 3))
    bs1_h = np.ascontiguousarray(bs1.reshape(KC, 128).T)
    bs2_h = np.ascontiguousarray(bs2.reshape(KC, 128).T)
    wc_h = np.ascontiguousarray(
        wc_mod.reshape(N, KC, 128, H).transpose(2, 0, 1, 3))

    use_bf16 = os.environ.get("KERNEL_DTYPE", "bf16") == "bf16"
    nc = _build(npos, has_bc1, bc2, use_bf16)
    if use_bf16:
        import ml_dtypes
        mmdt = ml_dtypes.bfloat16
    else:
        mmdt = np.float32

    in_maps = []
    for c in range(CORES):
        xc = x[c * BLOC:(c + 1) * BLOC]                   # [128, D, N]
        # token t = n*BLOC + b ; xs[kp, ki, t] = x[b, ki*128+kp, n]
        xs_h = np.ascontiguousarray(
            xc.transpose(1, 2, 0)                          # [D, N, BLOC]
            .reshape(KC, 128, T)
            .transpose(1, 0, 2))
        m = {
            "xs": xs_h.astype(mmdt),
            "ws1": ws1_h.astype(mmdt), "ws2": ws2_h.astype(mmdt),
            "bs1": bs1_h, "bs2": bs2_h,
            "wc": wc_h.astype(mmdt),
        }
        if has_bc1:
            m["bc1m"] = np.ascontiguousarray(bc1_mod)
        in_maps.append(m)

    res = run_bass_kernel_spmd(
        nc, in_maps, core_ids=list(range(CORES)),
        trace=bool(int(os.environ.get("KERNEL_TRACE", "0"))),
    )
    LAST_RESULTS = res

    out = np.empty((B, 1, N), dtype=np.float32)
    full = np.empty((N * B, 1, N), dtype=np.float32)
    fullv = full.reshape(N, CORES, BLOC, N)
    for c in range(CORES):
        r = res.results[c]
        out[c * BLOC:(c + 1) * BLOC, 0, :] = r["probs"]
        fullv[:, c, :, :] = r["flog"].reshape(N, BLOC, N)
    return out, full


# revision 24
# speedup vs baseline: 1.6029x; 1.0137x over previous
"""Trainium2 Bass kernel for nn_MultiLabelClassifier.

Reference computation (B=1024, D=512, N=16, H=512):
    xb     = transpose(x, (0,2,1))                        # [B, N, D]
    h      = lrelu(xb @ Ws1 + bs1)                        # shared MLP
    shared = lrelu(h @ Ws2 + bs2)                         # [B, N, D]
    out    = sigmoid(per-branch classifier on own slice)  # [B, 1, N]
    full   = all 16 classifiers on all N*B tokens         # [N*B, 1, N]

Key identities used:
  1. `out` is the diagonal of `full`: out[b, n] = sigmoid(full[n*B+b, n]),
     so only the full pass is computed on device.
  2. The second classifier layer collapses into the first via positive
     homogeneity of leaky-relu:
         sum_h w_h * lrelu(z_h)                     (w = Wc2[n], z = x@Wc1[n]+bc1[n])
       = sum_{w_h>=0} prelu(|w_h| z_h, a=0.1)       (columns scaled by w)
       + sum_{w_h<0}  prelu(-0.1|w_h| z_h, a=10)    (columns scaled by 0.1*w)
     Host permutes the H columns of each branch's Wc1 so the w>=0 group comes
     first, and pre-scales columns; on device the H-reduction is then just the
     ScalarEngine's free-axis accumulate (accum_out) on two Prelu activations.

Sharding: pure data-parallel over batch, 128 batch rows per core, 8 cores.
Matmuls run as float32r (fp32 data, 1 cycle/row at free-dim >= 256).
"""

import os
import numpy as np

B, D, N, H = 1024, 512, 16, 512
CORES = 8
BLOC = B // CORES           # batch rows per core
T = N * BLOC                # tokens per core (token t = n*BLOC + b)
KC = D // 128               # contraction chunks
FCH = 512                   # token free-chunk for stages 1-2
NF = T // FCH               # number of free chunks
TPF = FCH // 128            # 128-token tiles per free chunk

# Stash of the last BassKernelResults (for test harness to read exec_time_ns).
LAST_RESULTS = None


def _build(npos, has_bc1, bc2, use_bf16):
    """Build the per-core Bass program (same program for every core)."""
    import concourse.bacc as bacc
    import concourse.tile as tile
    from concourse import mybir

    f32 = mybir.dt.float32
    f32r = mybir.dt.bfloat16 if use_bf16 else mybir.dt.float32r
    bf = mybir.dt.bfloat16 if use_bf16 else mybir.dt.float32
    A = mybir.ActivationFunctionType

    has_bc2 = bool(np.any(bc2 != 0.0))

    pos_max = max(1, int(max(npos)))
    neg_max = max(1, int(H - min(npos)))

    nc = bacc.Bacc(trn_type="TRN2")

    xs_d = nc.dram_tensor("xs", [128, KC, T], f32r, kind="ExternalInput")
    ws1_d = nc.dram_tensor("ws1", [128, KC, KC, 128], f32r, kind="ExternalInput")
    ws2_d = nc.dram_tensor("ws2", [128, KC, KC, 128], f32r, kind="ExternalInput")
    bs1_d = nc.dram_tensor("bs1", [128, KC], f32, kind="ExternalInput")
    bs2_d = nc.dram_tensor("bs2", [128, KC], f32, kind="ExternalInput")
    wc_d = nc.dram_tensor("wc", [128, N, KC, H], f32r, kind="ExternalInput")
    if has_bc1:
        bc1_d = nc.dram_tensor("bc1m", [N, H], f32, kind="ExternalInput")
    flog_d = nc.dram_tensor("flog", [T, N], f32, kind="ExternalOutput")
    probs_d = nc.dram_tensor("probs", [BLOC, N], f32, kind="ExternalOutput")

    with tile.TileContext(nc) as tc:
        with (
            tc.tile_pool(name="consts", bufs=1) as consts,
            tc.tile_pool(name="xs", bufs=4) as xs_pool,
            tc.tile_pool(name="h", bufs=1) as h_pool,
            tc.tile_pool(name="sh", bufs=2) as sh_pool,
            tc.tile_pool(name="scr", bufs=1) as scr_pool,
            tc.tile_pool(name="dv", bufs=4) as dv_pool,
            tc.tile_pool(name="acc", bufs=12) as acc_pool,
            tc.tile_pool(name="zed", bufs=2) as z_pool,
            tc.tile_pool(name="outs", bufs=1) as out_pool,
            tc.tile_pool(name="ps12", bufs=2, space="PSUM") as ps12,
            tc.tile_pool(name="ps3", bufs=3, space="PSUM") as ps3,
        ):
            xs_tiles = []
            for f in range(NF):
                xs_t = xs_pool.tile([128, KC, FCH], f32r)
                nc.sync.dma_start(
                    out=xs_t[:], in_=xs_d[:, :, f * FCH:(f + 1) * FCH])
                xs_tiles.append(xs_t)
            ws1_sb = consts.tile([128, KC, KC, 128], f32r)
            nc.scalar.dma_start(out=ws1_sb[:], in_=ws1_d[:])
            ws2_sb = consts.tile([128, KC, KC, 128], f32r)
            nc.scalar.dma_start(out=ws2_sb[:], in_=ws2_d[:])
            bs1_sb = consts.tile([128, KC], f32)
            nc.scalar.dma_start(out=bs1_sb[:], in_=bs1_d[:])
            bs2_sb = consts.tile([128, KC], f32)
            nc.scalar.dma_start(out=bs2_sb[:], in_=bs2_d[:])
            if has_bc1:
                bc1_sb = consts.tile([128, N, H], f32)
                nc.sync.dma_start(
                    out=bc1_sb[:], in_=bc1_d[:].partition_broadcast(128)
                )
            wc_sb = consts.tile([128, N, KC, H], f32r)
            for n in range(N):
                nc.sync.dma_start(out=wc_sb[:, n], in_=wc_d[:, n])

            logits_sb = out_pool.tile([128, N, N], f32)   # [b, ttile, n]
            probs_sb = out_pool.tile([128, N], f32)

            for f in range(NF):
                xs_t = xs_tiles[f]

                # stage 1: h = prelu(Ws1^T x + bs1)
                h_t = h_pool.tile([128, KC, FCH], f32r)
                for oi in range(KC):
                    ps = ps12.tile([128, FCH], f32)
                    for ki in range(KC):
                        nc.tensor.matmul(
                            ps[:],
                            ws1_sb[:, ki, oi, :],
                            xs_t[:, ki, :],
                            start=(ki == 0),
                            stop=(ki == KC - 1),
                        )
                    nc.scalar.activation(
                        h_t[:, oi, :], ps[:], A.Prelu,
                        bias=bs1_sb[:, oi:oi + 1], scale=1.0, alpha=0.1,
                    )

                # stage 2: shared = prelu(Ws2^T h + bs2)
                sh_t = sh_pool.tile([128, KC, FCH], f32r)
                for oi in range(KC):
                    ps = ps12.tile([128, FCH], f32)
                    for ki in range(KC):
                        nc.tensor.matmul(
                            ps[:],
                            ws2_sb[:, ki, oi, :],
                            h_t[:, ki, :],
                            start=(ki == 0),
                            stop=(ki == KC - 1),
                        )
                    nc.scalar.activation(
                        sh_t[:, oi, :], ps[:], A.Prelu,
                        bias=bs2_sb[:, oi:oi + 1], scale=1.0, alpha=0.1,
                    )

                # stage 3: all 16 classifiers on this chunk's 4 token-tiles.
                # Branches are processed in pairs sharing one 2-bank psum
                # tile laid out [pos_e|neg_e|neg_o|pos_o], so ScalarE needs
                # only 3 Prelu ranges per pair and VectorE one reduce.
                for tt in range(TPF):
                    t = f * TPF + tt          # global token tile == branch id
                    c0 = tt * 128
                    for p2 in range(N // 2):
                        ne, no = 2 * p2, 2 * p2 + 1
                        ps = ps3.tile([128, 2 * H], f32)
                        for j, n in ((0, ne), (1, no)):
                            for ki in range(KC):
                                nc.tensor.matmul(
                                    ps[:, j * H:(j + 1) * H],
                                    sh_t[:, ki, c0:c0 + 128],
                                    wc_sb[:, n, ki, :],
                                    start=(ki == 0),
                                    stop=(ki == KC - 1),
                                )
                        if has_bc1:
                            z_t = z_pool.tile([128, 2 * H], f32)
                            nc.vector.tensor_add(
                                z_t[:, :H], ps[:, :H], bc1_sb[:, ne, :])
                            nc.vector.tensor_add(
                                z_t[:, H:], ps[:, H:], bc1_sb[:, no, :])
                            src = z_t
                        else:
                            src = ps
                        s1 = int(npos[ne])
                        s2 = 2 * H - int(npos[no])
                        ucp = dv_pool.tile([128, 2 * H], bf, tag="ucp")
                        if s1 > 0:
                            nc.scalar.activation(
                                ucp[:, :s1], src[:, :s1], A.Prelu,
                                bias=0.0, scale=1.0, alpha=0.1)
                        if s2 > s1:
                            nc.scalar.activation(
                                ucp[:, s1:s2], src[:, s1:s2], A.Prelu,
                                bias=0.0, scale=1.0, alpha=10.0)
                        if s2 < 2 * H:
                            nc.scalar.activation(
                                ucp[:, s2:], src[:, s2:], A.Prelu,
                                bias=0.0, scale=1.0, alpha=0.1)
                        nc.vector.tensor_reduce(
                            out=logits_sb[:, t, ne:ne + 2],
                            in_=ucp[:].rearrange("p (two h) -> p two h", two=2),
                            axis=mybir.AxisListType.X,
                            op=mybir.AluOpType.add,
                        )
                    nc.scalar.activation(
                        probs_sb[:, t:t + 1], logits_sb[:, t, t:t + 1],
                        A.Sigmoid,
                    )
                    nc.sync.dma_start(
                        out=flog_d[t * 128:(t + 1) * 128, :],
                        in_=logits_sb[:, t, :],
                    )
            nc.sync.dma_start(out=probs_d[:], in_=probs_sb[:])

    nc.compile()
    return nc


def kernel(x, Ws1, bs1, Ws2, bs2, Wc1, bc1, Wc2, bc2):
    global LAST_RESULTS
    from concourse.bass_utils import run_bass_kernel_spmd

    x = np.ascontiguousarray(np.asarray(x, dtype=np.float32))
    Ws1 = np.asarray(Ws1, dtype=np.float32)
    bs1 = np.asarray(bs1, dtype=np.float32)
    Ws2 = np.asarray(Ws2, dtype=np.float32)
    bs2 = np.asarray(bs2, dtype=np.float32)
    Wc1 = np.asarray(Wc1, dtype=np.float32)
    bc1 = np.asarray(bc1, dtype=np.float32)
    Wc2 = np.asarray(Wc2, dtype=np.float32)
    bc2 = np.asarray(bc2, dtype=np.float32)

    # ---- host-side weight preprocessing (O(weights) only) ----
    # Per branch: put w>=0 columns first, scale pos cols by w, neg by 0.1*w
    # (see module docstring identity 2).
    npos = np.zeros(N, dtype=np.int64)
    wc_mod = np.empty_like(Wc1)               # [N, D, H]
    bc1_mod = np.empty_like(bc1)              # [N, H]
    for n in range(N):
        w = Wc2[n]
        pos = np.flatnonzero(w >= 0.0)
        neg = np.flatnonzero(w < 0.0)
        npos[n] = len(pos)
        if n % 2 == 0:       # even branch: [pos | neg]
            scale = np.concatenate([w[pos], 0.1 * w[neg]])
            perm = np.concatenate([pos, neg])
        else:                # odd branch: [neg | pos] (pairs with even's neg)
            scale = np.concatenate([0.1 * w[neg], w[pos]])
            perm = np.concatenate([neg, pos])
        wc_mod[n] = Wc1[n][:, perm] * scale[None, :]
        bc1_mod[n] = bc1[n][perm] * scale

    has_bc1 = bool(np.any(bc1_mod != 0.0))

    # ---- reshape to device layouts ----
    ws1_h = np.ascontiguousarray(
        Ws1.reshape(KC, 128, KC, 128).transpose(1, 0, 2, 3))
    ws2_h = np.ascontiguousarray(
        Ws2.reshape(KC, 128, KC, 128).transpose(1, 0, 2, 3))
    bs1_h = np.ascontiguousarray(bs1.reshape(KC, 128).T)
    bs2_h = np.ascontiguousarray(bs2.reshape(KC, 128).T)
    wc_h = np.ascontiguousarray(
        wc_mod.reshape(N, KC, 128, H).transpose(2, 0, 1, 3))

    use_bf16 = os.environ.get("KERNEL_DTYPE", "bf16") == "bf16"
    nc = _build(npos, has_bc1, bc2, use_bf16)
    if use_bf16:
        import ml_dtypes
        mmdt = ml_dtypes.bfloat16
    else:
        mmdt = np.float32

    in_maps = []
    for c in range(CORES):
        xc = x[c * BLOC:(c + 1) * BLOC]                   # [128, D, N]
        # token t = n*BLOC + b ; xs[kp, ki, t] = x[b, ki*128+kp, n]
        xs_h = np.ascontiguousarray(
            xc.transpose(1, 2, 0)                          # [D, N, BLOC]
            .reshape(KC, 128, T)
            .transpose(1, 0, 2))
        m = {
            "xs": xs_h.astype(mmdt),
            "ws1": ws1_h.astype(mmdt), "ws2": ws2_h.astype(mmdt),
            "bs1": bs1_h, "bs2": bs2_h,
            "wc": wc_h.astype(mmdt),
        }
        if has_bc1:
            m["bc1m"] = np.ascontiguousarray(bc1_mod)
        in_maps.append(m)

    res = run_bass_kernel_spmd(
        nc, in_maps, core_ids=list(range(CORES)),
        trace=bool(int(os.environ.get("KERNEL_TRACE", "0"))),
    )
    LAST_RESULTS = res

    out = np.empty((B, 1, N), dtype=np.float32)
    full = np.empty((N * B, 1, N), dtype=np.float32)
    fullv = full.reshape(N, CORES, BLOC, N)
    for c in range(CORES):
        r = res.results[c]
        out[c * BLOC:(c + 1) * BLOC, 0, :] = r["probs"]
        fullv[:, c, :, :] = r["flog"].reshape(N, BLOC, N)
    return out, full
